# revision 24
# baseline (speedup 1.0000x reference)
"""GAT layer (MixGATLayer) Trainium2 kernel.

Strategy (8 NeuronCores, SPMD, zero collectives):
  - Host: drop self-loop edges (handled separately), sort remaining edges
    by dst, shard by dst-node range (6250 nodes/core); fold att_src/att_dst
    into the GEMM weights: W_ext = [W | W @ A_src | W @ A_dst] (128 x 264).
  - Phase 1 (per core, replicated): GA = x @ W_ext -> DRAM [N, 320] f32
    (columns: h[256] | a_src[4] | a_dst[4] | pad; row stride 1280B for
    dma_gather's 256B-multiple constraint). 4 row-tiles batched per DMA;
    PSUM evacuated on the scalar engine.
  - Phase 2 (per core, its dst range): per 128-node strip,
      * self-loop chunk: sequential DMA of ga[strip] (no gather), identity
        one-hot matmul;
      * real edges: gather GA rows by src via gpsimd dma_gather (int16
        indices; edges split into low/high 32768-row halves of GA), gather
        per-edge a_dst windows from the strip rows; 4 SWDGE queues.
      * ex = exp(leaky_relu(a_src+a_dst)) (Lrelu+Exp on scalar engine);
        scale messages by ex (vector); aggregate each 128-edge chunk with a
        one-hot matmul accumulating into a strip PSUM [128, 260] whose last
        4 columns accumulate the softmax denominators; normalize, add bias,
        apply 0.5*z + 0.5*elu(z), store.

  Softmax max-subtraction is skipped (mathematically a no-op for the
  result; logits are O(+-6) so exp() cannot overflow in fp32).
"""

import numpy as np

_P = 128
_H, _F = 4, 64
_HF = _H * _F            # 256
_GAW = 320               # h[256] | a_src[4] | a_dst[4] | pad[56]
_ACOL = _HF              # a_src column offset in GA
_NEG = 0.2               # leaky_relu slope
_N_CORES = 8
_H0 = 32768              # int16-index half split of GA rows
_CAP = 6                # max chunks (of 128 edges) per dma_gather op
_NQ = 4                  # SWDGE queues
_SCRATCH = 16384         # dynamic dma scratch (ring capacity)
_PB = 8                  # phase-1 row-tiles per batched DMA
_GATH_BUFS = 3           # gather tile double/triple buffering


def _wrap16(idx_flat):
    # dma_gather index layout: idx i at partition i%16, column i//16,
    # replicated across the 8 gpsimd cores' partition groups.
    n = idx_flat.shape[0]
    assert n % 16 == 0
    arr = idx_flat.reshape(n // 16, 16).T  # [16, n//16]
    return np.tile(arr, (8, 1))            # [128, n//16]


def _preprocess(x, edge_index, W, att_src, att_dst, bias, n_cores=_N_CORES,
                cap=_CAP):
    x = np.asarray(x, np.float32)
    N, in_dim = x.shape
    assert in_dim == _P, "GEMM tiling assumes in_dim == 128"
    npc = N // n_cores
    assert npc * n_cores == N

    src = np.asarray(edge_index[0]).astype(np.int64)
    dst = np.asarray(edge_index[1]).astype(np.int64)
    # self-loops handled as a dedicated per-strip chunk; drop any explicit
    # duplicates from the edge list? PyG adds loops unconditionally on top
    # of existing edges, so keep all input edges and ADD the loop chunk.
    order = np.argsort(dst, kind="stable")
    src_s = src[order]
    dst_s = dst[order]

    S = (npc + _P - 1) // _P
    eA = [[None] * S for _ in range(n_cores)]
    eB = [[None] * S for _ in range(n_cores)]
    KA = np.zeros((n_cores, S), np.int64)
    KB = np.zeros((n_cores, S), np.int64)
    for c in range(n_cores):
        base = c * npc
        for s in range(S):
            lo_n = base + s * _P
            hi_n = min(base + (s + 1) * _P, base + npc)
            lo_e = np.searchsorted(dst_s, lo_n, side="left")
            hi_e = np.searchsorted(dst_s, hi_n, side="left")
            es = ((src_s[lo_e:hi_e] - base) % N).astype(np.int32)
            ed = dst_s[lo_e:hi_e]
            m = es < _H0
            dl = (ed - lo_n).astype(np.float32)
            eA[c][s] = (es[m], dl[m])
            eB[c][s] = (es[~m] - _H0, dl[~m])
            KA[c, s] = -(-len(eA[c][s][0]) // _P)
            KB[c, s] = -(-len(eB[c][s][0]) // _P)
    KAs = [int(k) for k in KA.max(axis=0)]
    KBs = [int(k) for k in KB.max(axis=0)]
    KS = [a + b for a, b in zip(KAs, KBs)]
    tot = int(sum(KS))
    off = np.concatenate([[0], np.cumsum(KS)]).astype(np.int64)

    def ops_for(k):
        return [(c0, min(c0 + cap, k)) for c0 in range(0, k, cap)] if k else []

    # merged per-strip index stream: per strip [128, Kc*18] int16 columns:
    #   [0 : 8Kc)    gidx (wrapped, per-op concatenated)
    #   [8Kc : 16Kc) a2idx (wrapped)
    #   [16Kc: 18Kc) dloc as f32 bitcast into int16 pairs
    totcol = int(sum(k * 18 for k in KS))
    idxs = np.zeros((n_cores, _P, totcol), np.int16)
    colo = np.concatenate([[0], np.cumsum([k * 18 for k in KS])]).astype(np.int64)
    for c in range(n_cores):
        for s in range(S):
            kA, kB, kc = KAs[s], KBs[s], KS[s]
            if kc == 0:
                continue
            sidx = np.zeros(kc * _P, np.int32)
            sdl = np.full(kc * _P, -1.0, np.float32)
            a_src_i, a_dl = eA[c][s]
            b_src_i, b_dl = eB[c][s]
            sidx[: len(a_src_i)] = a_src_i
            sdl[: len(a_src_i)] = a_dl
            sidx[kA * _P: kA * _P + len(b_src_i)] = b_src_i
            sdl[kA * _P: kA * _P + len(b_src_i)] = b_dl
            co = int(colo[s])
            # gather idx: wrapped per op, concatenated columns
            cols = []
            for (c0, c1) in ops_for(kA) + [(kA + a, kA + b) for (a, b) in ops_for(kB)]:
                cols.append(_wrap16(sidx[c0 * _P: c1 * _P].astype(np.int16)))
            idxs[c, :, co: co + kc * 8] = np.concatenate(cols, axis=1)
            # a2 idx (strip-local dst ids; pads -> 0)
            a2 = np.maximum(sdl, 0.0).astype(np.int16)
            cols = [_wrap16(a2[c0 * _P: c1 * _P]) for (c0, c1) in ops_for(kc)]
            idxs[c, :, co + kc * 8: co + kc * 16] = np.concatenate(cols, axis=1)
            # dloc tile layout [128, kc] f32: tile[p, k] = slot k*128+p
            dlt = np.ascontiguousarray(sdl.reshape(kc, _P).T)  # [128, kc] f32
            idxs[c, :, co + kc * 16: co + kc * 18] = (
                dlt.view(np.int16).reshape(_P, kc * 2)
            )

    Wf = np.asarray(W, np.float32)
    a_s = np.asarray(att_src, np.float32)
    a_d = np.asarray(att_dst, np.float32)
    Wsrc = np.einsum("ihf,hf->ih", Wf.reshape(in_dim, _H, _F), a_s).astype(np.float32)
    Wdst = np.einsum("ihf,hf->ih", Wf.reshape(in_dim, _H, _F), a_d).astype(np.float32)
    wext = np.ascontiguousarray(np.concatenate([Wf, Wsrc, Wdst], axis=1))
    xT = np.stack(
        [np.ascontiguousarray(np.roll(x, -c * npc, axis=0).T)
         for c in range(n_cores)]
    )  # [n_cores, in_dim, N]
    biasb = np.ascontiguousarray(
        np.tile(np.asarray(bias, np.float32)[None, :], (_P, 1)))
    iota = np.ascontiguousarray(
        np.tile(np.arange(_P, dtype=np.float32)[None, :], (_P, 1)))
    ident = np.eye(_P, dtype=np.float32)
    # single combined input per core (per-invocation overhead scales with
    # input-tensor COUNT on this runtime):
    # [wext 264 | biasb 256 | iota 128 | ident 128 | idxs totcol/2 | xT N]
    assert totcol % 2 == 0
    combo = np.concatenate(
        [
            np.broadcast_to(wext[None], (n_cores, _P, _HF + 8)),
            np.broadcast_to(biasb[None], (n_cores, _P, _HF)),
            np.broadcast_to(iota[None], (n_cores, _P, _P)),
            np.broadcast_to(ident[None], (n_cores, _P, _P)),
            idxs.view(np.float32),
            xT,
        ],
        axis=2,
    ).astype(np.float32)
    combo = np.ascontiguousarray(combo)
    return dict(
        KAs=KAs, KBs=KBs, KS=KS, cap=cap, combo=combo,
        N=N, npc=npc, S=S,
    )


def _dma_gather_raw(g, out_ap, in_ap, idxs_ap, num_idxs, elem_size, elem_step,
                    queue_num=0):
    """dma_gather without the 256B elem_size restriction (transpose-only in
    the Q7 ucode; the non-transpose descriptor path takes raw byte sizes).
    The row stride (elem_step) must still encode as a multiple of 256B."""
    import concourse.mybir as mybir

    g._assert_queue_num(queue_num)
    dsz = mybir.dt.size(in_ap.dtype)
    stride_bytes = elem_step * dsz
    assert stride_bytes % 256 == 0 and stride_bytes // 256 < 256
    assert in_ap.ap[0][0] == elem_step
    assert in_ap.ap[-1][1] == out_ap.ap[-1][1] == elem_size
    assert idxs_ap.dtype == mybir.dt.int16
    return g.add_instruction(
        mybir.InstDMAGatherAnt(
            name=g.bass.get_next_instruction_name(),
            ins=[
                *g.lower_ap_dma(in_ap, for_custom_bir_dma=True),
                g.lower_ap(idxs_ap),
                g.lower_val_access(g.to_reg(num_idxs)),
            ],
            outs=[g.lower_ap(out_ap)],
            transpose=False,
            num_idxs=num_idxs,
            elem_size=elem_size,
            stride_bytes_256=stride_bytes // 256,
            gen_mode=0,
            single_packet=True,
            queue_num=queue_num,
            sbuf_tokens_per_rank=0,
            sbuf_free_dim_per_rank=0,
            sbuf_free_dim_pad_per_rank=0,
            sbuf_byte_offset=0,
        )
    )


def _build(KAs, KBs, N, npc, cap=_CAP, p1_only=False, full_rows=False):
    import concourse.bacc as bacc
    import concourse.bass as bass
    import concourse.mybir as mybir
    import concourse.tile as tile

    f32 = mybir.dt.float32
    i16 = mybir.dt.int16
    AF = mybir.ActivationFunctionType
    OP = mybir.AluOpType

    S = len(KAs)
    KS = [a + b for a, b in zip(KAs, KBs)]
    totcol = int(sum(k * 18 for k in KS))
    colo = np.concatenate([[0], np.cumsum([k * 18 for k in KS])]).astype(np.int64)

    nc = bacc.Bacc(
        "TRN2", target_bir_lowering=False, debug=False, enable_asserts=False,
        num_swdge_queues=_NQ, dynamic_dma_scratch_size=_SCRATCH,
        enable_partition_id=False,
    )
    assert totcol % 2 == 0
    combw = (_HF + 8) + _HF + _P + _P + totcol // 2 + N
    combo = nc.dram_tensor("combo", [_P, combw], f32,
                           kind="ExternalInput").ap()
    o = 0
    wext = combo[:, o:o + _HF + 8]; o += _HF + 8
    biasb = combo[:, o:o + _HF]; o += _HF
    iota = combo[:, o:o + _P]; o += _P
    ident = combo[:, o:o + _P]; o += _P
    idxs = combo[:, o:o + totcol // 2].bitcast(i16); o += totcol // 2
    xT = combo[:, o:o + N]
    out = nc.dram_tensor("out", [npc, _HF], f32, kind="ExternalOutput").ap()

    def ops_for(k):
        return [(c0, min(c0 + cap, k)) for c0 in range(0, k, cap)] if k else []

    qn = [0]

    def next_q():
        q = qn[0]
        qn[0] = (q + 1) % _NQ
        return q

    with tile.TileContext(nc) as tc:
        with (
            tc.tile_pool(name="dram", bufs=1, space="DRAM") as dpool,
            tc.tile_pool(name="const", bufs=1) as cpool,
            tc.tile_pool(name="sb", bufs=3) as pool,
            tc.tile_pool(name="gath", bufs=_GATH_BUFS) as gpool,
            tc.tile_pool(name="ps1", bufs=4, space="PSUM") as ps1pool,
            tc.tile_pool(name="ps2", bufs=4, space="PSUM") as ps2pool,
        ):
            ga = dpool.tile([N, _GAW], f32)
            wt = cpool.tile([_P, _HF + 8], f32)
            nc.sync.dma_start(out=wt[:], in_=wext)
            it = cpool.tile([_P, _P], f32)
            nc.sync.dma_start(out=it[:], in_=iota)
            idt = cpool.tile([_P, _P], f32)
            nc.sync.dma_start(out=idt[:], in_=ident)
            bt = cpool.tile([_P, _HF], f32)
            nc.sync.dma_start(out=bt[:], in_=biasb)

            # ---- phase 1: GA[:, 0:264] = x @ W_ext (batched) ----
            ntile = (N + _P - 1) // _P
            nb = (ntile + _PB - 1) // _PB
            for b in range(nb):
                t0 = b * _PB
                t1 = min(t0 + _PB, ntile)
                r0 = t0 * _P
                rows_b = min(_PB * _P, N - r0)
                bt_n = t1 - t0
                xt_t = pool.tile([_P, _PB * _P], f32, tag="xt")
                nc.scalar.dma_start(
                    out=xt_t[:, :rows_b], in_=xT[:, r0:r0 + rows_b]
                )
                # full_rows: evb covers the full 320-float GA row (cols
                # 264:320 stale, never consumed) so the GA store DMA is
                # hole-free; otherwise write 264-float rows strided by 320.
                gw = _GAW if full_rows else _HF + 8
                evb = pool.tile([_P, _PB, gw], f32, tag="evb")
                for t in range(bt_n):
                    rows = min(_P, N - (r0 + t * _P))
                    hps = ps1pool.tile([_P, _HF + 8], f32, tag="hps")
                    nc.tensor.matmul(
                        hps[:rows, :],
                        lhsT=xt_t[:, t * _P: t * _P + rows], rhs=wt[:],
                        start=True, stop=True,
                    )
                    nc.scalar.activation(
                        out=evb[:rows, t, 0:_HF + 8], in_=hps[:rows, :],
                        func=AF.Copy
                    )
                if rows_b == _PB * _P:
                    nc.sync.dma_start(
                        out=ga[r0:r0 + rows_b, 0:gw]
                        .rearrange("(t p) c -> p t c", p=_P),
                        in_=evb[:],
                    )
                else:
                    for t in range(bt_n):
                        rows = min(_P, N - (r0 + t * _P))
                        nc.sync.dma_start(
                            out=ga[r0 + t * _P:r0 + t * _P + rows, 0:gw],
                            in_=evb[:rows, t, :],
                        )

            tc.strict_bb_all_engine_barrier()

            # ---- phase 2: edge aggregation per 128-dst strip ----
            gaA = ga[0:min(_H0, N), :]
            gaB = ga[_H0:N, :] if N > _H0 else None
            if p1_only:
                nc.sync.dma_start(out=out[0:_P, :], in_=ga[0:_P, 0:_HF])
            for s in (range(0) if p1_only else range(S)):
                kA, kB = KAs[s], KBs[s]
                Kc = kA + kB
                r0 = s * _P
                rows = min(_P, npc - r0)
                co = int(colo[s])

                ixt = pool.tile([_P, Kc * 18], i16, tag="ixt")
                nc.scalar.dma_start(out=ixt[:], in_=idxs[:, co:co + Kc * 18])
                gixt = ixt[:, 0:Kc * 8]
                a2xt = ixt[:, Kc * 8:Kc * 16]
                dl = ixt[:, Kc * 16:Kc * 18].bitcast(f32)  # [128, Kc]

                # self-loop chunk: strip rows, sequential load
                gs = pool.tile([_P, _HF + 8], f32, tag="gs")
                if rows < _P:
                    nc.vector.memset(gs[:], 0.0)
                nc.scalar.dma_start(
                    out=gs[:rows, :], in_=ga[r0:r0 + rows, 0:_HF + 8]
                )

                # gather only the useful 260 floats (h + a_src) of each
                # 320-float GA row; a_dst of the src node is not needed
                gat = gpool.tile([_P, Kc, _HF + _H], f32, tag="gat")
                for (c0, c1), gv in (
                    [((a, b), gaA) for (a, b) in ops_for(kA)]
                    + [((kA + a, kA + b), gaB) for (a, b) in ops_for(kB)]
                ):
                    n = (c1 - c0) * _P
                    _dma_gather_raw(
                        nc.gpsimd, out_ap=gat[:, c0:c1, :],
                        in_ap=gv[:, 0:_HF + _H],
                        idxs_ap=gixt[:, c0 * 8:c1 * 8],
                        num_idxs=n, elem_size=_HF + _H, elem_step=_GAW,
                        queue_num=next_q(),
                    )
                a2t = pool.tile([_P, Kc, 8], f32, tag="a2t")
                gstrip = ga[r0:r0 + rows, _ACOL:_ACOL + 8]
                for (c0, c1) in ops_for(Kc):
                    n = (c1 - c0) * _P
                    _dma_gather_raw(
                        nc.gpsimd, out_ap=a2t[:, c0:c1, :], in_ap=gstrip,
                        idxs_ap=a2xt[:, c0 * 8:c1 * 8],
                        num_idxs=n, elem_size=8, elem_step=_GAW,
                        queue_num=next_q(),
                    )

                # logits: ex = exp(leaky_relu(a_src + a_dst)); ex overwrites
                # the a_src columns of gat so each chunk's matmul rhs
                # [h*ex | ex] is one contiguous [128, 260] slice.
                u = pool.tile([_P, Kc, _H], f32, tag="u")
                nc.vector.tensor_tensor(
                    out=u[:], in0=gat[:, :, _ACOL:_ACOL + _H],
                    in1=a2t[:, :, 4:8], op=OP.add,
                )
                ul = pool.tile([_P, Kc, _H], f32, tag="ul")
                nc.scalar.activation(out=ul[:], in_=u[:], func=AF.Prelu,
                                     alpha=_NEG)
                nc.scalar.activation(
                    out=gat[:, :, _ACOL:_ACOL + _H], in_=ul[:], func=AF.Exp
                )
                # self logits
                us = pool.tile([_P, _H], f32, tag="us")
                nc.vector.tensor_tensor(
                    out=us[:], in0=gs[:, _ACOL:_ACOL + _H],
                    in1=gs[:, _ACOL + _H:_ACOL + 2 * _H], op=OP.add,
                )
                uls = pool.tile([_P, _H], f32, tag="uls")
                nc.scalar.activation(out=uls[:], in_=us[:], func=AF.Prelu,
                                     alpha=_NEG)
                nc.scalar.activation(
                    out=gs[:, _ACOL:_ACOL + _H], in_=uls[:], func=AF.Exp
                )

                msg = gat[:, :, 0:_HF].rearrange("p k (h f) -> p k h f", h=_H)
                exv = (
                    gat[:, :, _ACOL:_ACOL + _H]
                    .rearrange("p k (h o) -> p k h o", o=1)
                    .to_broadcast([_P, Kc, _H, _F])
                )
                nc.vector.tensor_tensor(out=msg, in0=msg, in1=exv, op=OP.mult)
                msgs = gs[:, 0:_HF].rearrange("p (h f) -> p h f", h=_H)
                exvs = (
                    gs[:, _ACOL:_ACOL + _H]
                    .rearrange("p (h o) -> p h o", o=1)
                    .to_broadcast([_P, _H, _F])
                )
                nc.vector.tensor_tensor(out=msgs, in0=msgs, in1=exvs, op=OP.mult)

                # all chunk one-hots in one DVE op: oh[p, k, c] = (dl[p,k]==c)
                oh = pool.tile([_P, Kc, _P], f32, tag="oh")
                nc.vector.tensor_tensor(
                    out=oh[:],
                    in0=dl.rearrange("p (k o) -> p k o", o=1)
                    .to_broadcast([_P, Kc, _P]),
                    in1=it[:].rearrange("p (o c) -> p o c", o=1)
                    .to_broadcast([_P, Kc, _P]),
                    op=OP.is_equal,
                )

                agg = ps2pool.tile([_P, _HF + _H], f32, tag="agg")
                nc.tensor.matmul(
                    agg[:], lhsT=idt[:], rhs=gs[:, 0:_HF + _H],
                    start=True, stop=(Kc == 0),
                )
                for k in range(Kc):
                    nc.tensor.matmul(
                        agg[:], lhsT=oh[:, k, :], rhs=gat[:, k, 0:_HF + _H],
                        start=False, stop=(k == Kc - 1),
                    )

                # normalize, bias, 0.5*z + 0.5*elu(z)
                rcp = pool.tile([_P, _H], f32, tag="rcp")
                nc.vector.reciprocal(rcp[:], agg[:, _HF:_HF + _H])
                z = pool.tile([_P, _HF], f32, tag="z")
                nc.vector.tensor_tensor(
                    out=z[:].rearrange("p (h f) -> p h f", h=_H),
                    in0=agg[:, 0:_HF].rearrange("p (h f) -> p h f", h=_H),
                    in1=rcp[:]
                    .rearrange("p (h o) -> p h o", o=1)
                    .to_broadcast([_P, _H, _F]),
                    op=OP.mult,
                )
                nc.vector.tensor_tensor(out=z[:], in0=z[:], in1=bt[:], op=OP.add)
                # y = relu(-z); em = exp(-y) = exp(min(z, 0))
                ym = pool.tile([_P, _HF], f32, tag="ym")
                nc.scalar.activation(out=ym[:], in_=z[:], func=AF.Relu,
                                     scale=-1.0)
                em = pool.tile([_P, _HF], f32, tag="em")
                nc.scalar.activation(out=em[:], in_=ym[:], func=AF.Exp,
                                     scale=-1.0)
                t3 = pool.tile([_P, _HF], f32, tag="t3")
                nc.vector.tensor_tensor(out=t3[:], in0=z[:], in1=em[:], op=OP.add)
                c2 = pool.tile([_P, _HF], f32, tag="c2")
                nc.scalar.activation(
                    out=c2[:], in_=t3[:], func=AF.Copy, scale=0.5, bias=-0.5
                )
                fo = pool.tile([_P, _HF], f32, tag="fo")
                nc.vector.tensor_tensor(out=fo[:], in0=z[:], in1=c2[:], op=OP.max)

                nc.sync.dma_start(out=out[r0:r0 + rows, :], in_=fo[:rows, :])

    nc.compile()
    return nc


def _in_map(pre, c):
    return {"combo": pre["combo"][c]}


def _run(nc, pre, n_cores=_N_CORES, trace=False, **kwargs):
    from concourse.bass_utils import run_bass_kernel_spmd

    in_maps = [_in_map(pre, c) for c in range(n_cores)]
    res = run_bass_kernel_spmd(
        nc, in_maps, list(range(n_cores)), trace=trace, **kwargs
    )
    full = np.concatenate(
        [res.results[c]["out"] for c in range(n_cores)], axis=0
    ).astype(np.float32)
    return full, res


def kernel(**inputs):
    pre = _preprocess(
        inputs["x"], inputs["edge_index"], inputs["W"],
        inputs["att_src"], inputs["att_dst"], inputs["bias"],
    )
    nc = _build(pre["KAs"], pre["KBs"], pre["N"], pre["npc"], cap=pre["cap"])
    full, _ = _run(nc, pre)
    return full
